# revision 1
# baseline (speedup 1.0000x reference)
"""DeepSpeed-style MLP block (pre-LN residual add + LN + GEMM+GELU + GEMM +
residual) for Trainium2, data-parallel over tokens across 8 NeuronCores.

Per-core pipeline (tokens sharded 8 x 4096, processed in 512-token blocks):
  r   = input + bias + residual                      (fp32, DVE)
  x0  = (r - mean(r)) * rsqrt(var(r) + eps)          (LN affine folded into W1/b1
                                                      on the host: W1' = gamma*W1,
                                                      b1' = b1 + beta @ W1)
  xT  = PE-transpose(x0)  [H on partitions]          (bf16, via identity matmul)
  hT  = gelu_tanh(W1'-chunks.T @ xT + b1')           (PE + ACT, bf16)
  out = hT-chunks.T @ W2 + (r + output_b)            (PE + DVE, fp32; the carry
                                                      r+output_b is kept as bf16
                                                      hi+lo halves, fp32-exact)

W1 is SBUF-resident (bf16); W2 streams as [128 x 4 x 512] super-chunks on the
HWDGE path. Both GEMMs use N=512 moving operands at the bf16 streaming rate
(~213 ns per 128x128x512 matmul). Emission is software-pipelined: block N+1's
loads/LN/transposes are emitted before block N's PSUM eviction so neither the
PE nor the DVE stream ever head-of-line blocks at a block boundary. DMA traffic
is split by engine: HWDGE/SP for input+weight streams, SWDGE/gpsimd for
broadcasts and output stores (keeps latency-critical loads unblocked).
Measured: ~0.97 ms HW exec, rel err ~1.1e-3 vs the fp32 reference.
"""

import sys

sys.path.insert(0, "/opt/trn_rl_repo")

import numpy as np
import ml_dtypes

import concourse.bass as bass
import concourse.mybir as mybir
import concourse.tile as tile
from concourse.masks import make_identity
from concourse.bass_utils import run_bass_kernel_spmd

AFT = mybir.ActivationFunctionType
FP32 = mybir.dt.float32
BF16 = mybir.dt.bfloat16

N_CORES = 8
B, S, H, I = 4, 8192, 1024, 4096
NTOK = B * S              # 32768 tokens total
T = NTOK // N_CORES       # 4096 tokens per core
TB = 512                  # tokens per block (moving free dim)
G = TB // 128             # 4 token sub-tiles per block
KH = H // 128             # 8 contraction chunks for GEMM1
MI = I // 128             # 32 I-chunks (GEMM1 out / GEMM2 contraction)
NH = H // 512             # 2 H output slices for GEMM2
EPS = 1e-5


def _split_multiwait_instructions(nc):
    """This walrus build accepts only ONE sync-wait command per instruction.
    Move extra waits onto fresh same-engine NOPs placed just before the
    offending instruction."""
    n_split = 0
    for f in nc.m.functions:
        for bb in f.blocks:
            insts = list(bb.instructions)
            new = []
            changed = False
            for inst in insts:
                si = inst.sync_info
                if si is not None and si.on_wait and len(si.on_wait) > 1:
                    waits = list(si.on_wait)
                    for w in waits[:-1]:
                        nop = mybir.InstNoOp(name=nc.get_next_instruction_name())
                        nop.engine = inst.engine
                        nop.sync_info = mybir.SyncInfo(on_wait=[w], on_update=[])
                        new.append(nop)
                        n_split += 1
                    si.on_wait = waits[-1:]
                    changed = True
                new.append(inst)
            if changed:
                bb.instructions = new
    return n_split


def _bcast_ap(ap, p=128):
    """AP view of a DRAM vector broadcast across p partitions."""
    return bass.AP(tensor=ap.tensor, offset=ap.offset, ap=[[0, p]] + list(ap.ap))


def _build(n_blocks=T // TB):
    nc = bass.Bass("TRN2")
    t_rows = n_blocks * TB
    xin = nc.declare_dram_parameter("xin", [t_rows, H], FP32, isOutput=False)
    res = nc.declare_dram_parameter("res", [t_rows, H], FP32, isOutput=False)
    w1 = nc.declare_dram_parameter("w1", [H, I], BF16, isOutput=False)
    w2 = nc.declare_dram_parameter("w2", [I, H], BF16, isOutput=False)
    biasv = nc.declare_dram_parameter("biasv", [H], FP32, isOutput=False)
    b1c = nc.declare_dram_parameter("b1c", [128, MI], FP32, isOutput=False)
    b2v = nc.declare_dram_parameter("b2v", [H], FP32, isOutput=False)
    out = nc.declare_dram_parameter("out", [t_rows, H], FP32, isOutput=True)

    with tile.TileContext(nc) as tc:
        with (
            tc.tile_pool(name="const", bufs=1) as const,
            tc.tile_pool(name="w2p", bufs=5) as w2p,
            tc.tile_pool(name="ing", bufs=2) as ing,
            tc.tile_pool(name="tmpg", bufs=2) as tmpg,
            tc.tile_pool(name="blk1", bufs=1) as blk1,
            tc.tile_pool(name="blk2", bufs=2) as blk2,
            tc.tile_pool(name="outp", bufs=4) as outp,
            tc.tile_pool(name="statp", bufs=4) as statp,
            tc.tile_pool(name="ps1", bufs=2, space="PSUM") as ps1,
            tc.tile_pool(name="ps2", bufs=4, space="PSUM") as ps2,
            tc.tile_pool(name="pst", bufs=2, space="PSUM") as pst,
        ):
            # ---- preload constants / weights (small consts first: the LN
            # chain needs bias_bc immediately; w1 is 8 MB and only needed
            # once GEMM1 of block 0 starts) ----
            bias_bc = const.tile([128, H], FP32)
            nc.gpsimd.dma_start(out=bias_bc, in_=_bcast_ap(biasv[:]))
            b2_bc = const.tile([128, H], FP32)
            nc.gpsimd.dma_start(out=b2_bc, in_=_bcast_ap(b2v[:]))
            b1_sb = const.tile([128, MI], FP32)
            nc.gpsimd.dma_start(out=b1_sb, in_=b1c[:, :])
            eps_t = const.tile([128, 1], FP32)
            nc.vector.memset(eps_t, EPS)
            ident = const.tile([128, 128], BF16)
            make_identity(nc, ident)

            def emit_ln(tb):
                """Load + pre-LN + LN + PE-transpose for block tb.
                Returns the block tiles used by the GEMM/evict stages."""
                t0 = tb * TB
                x0 = blk1.tile([128, G, H], BF16, name=f"x0_{tb}", tag="x0")
                xT = blk1.tile([128, KH, TB], BF16, name=f"xT_{tb}", tag="xT")
                # r + output_b carried to the final add as bf16 hi + lo halves
                # (sum is fp32-exact to ~2^-17 relative)
                r_hi = blk2.tile([128, G, H], BF16, name=f"rhi_{tb}", tag="rhi")
                r_lo = blk2.tile([128, G, H], BF16, name=f"rlo_{tb}", tag="rlo")
                for g in range(G):
                    ra, rb = t0 + g * 128, t0 + (g + 1) * 128
                    xin_g = ing.tile([128, H], FP32, name=f"xin_{tb}_{g}", tag="xin")
                    res_g = ing.tile([128, H], FP32, name=f"res_{tb}_{g}", tag="res")
                    nc.sync.dma_start(out=xin_g, in_=xin[ra:rb, :])
                    nc.sync.dma_start(out=res_g, in_=res[ra:rb, :])
                    tmp = tmpg.tile([128, H], FP32, name=f"tmp_{tb}_{g}", tag="tmp")
                    nc.vector.tensor_add(out=tmp, in0=xin_g, in1=res_g)
                    nc.vector.tensor_add(out=tmp, in0=tmp, in1=bias_bc)
                    stats = statp.tile([128, 2, 6], FP32, name=f"st_{tb}_{g}", tag="stats")
                    tmp_r = tmp.rearrange("p (s d) -> p s d", s=2)
                    for s_ in range(2):
                        nc.vector.bn_stats(out=stats[:, s_, :], in_=tmp_r[:, s_, :])
                    mv = statp.tile([128, 2], FP32, name=f"mv_{tb}_{g}", tag="mv")
                    nc.vector.bn_aggr(out=mv, in_=stats)
                    rstd = statp.tile([128, 1], FP32, name=f"rs_{tb}_{g}", tag="rstd")
                    nc.scalar.activation(
                        out=rstd, in_=mv[:, 1:2], func=AFT.Sqrt, bias=eps_t, scale=1.0
                    )
                    nc.vector.reciprocal(out=rstd, in_=rstd)
                    nc.vector.tensor_scalar(
                        out=x0[:, g, :],
                        in0=tmp,
                        scalar1=mv[:, 0:1],
                        scalar2=rstd,
                        op0=mybir.AluOpType.subtract,
                        op1=mybir.AluOpType.mult,
                    )
                    nc.vector.tensor_add(out=tmp, in0=tmp, in1=b2_bc)
                    nc.vector.tensor_copy(out=r_hi[:, g, :], in_=tmp)
                    nc.vector.tensor_sub(out=r_lo[:, g, :], in0=tmp, in1=r_hi[:, g, :])
                    for k in range(KH):
                        pt = pst.tile([128, 128], BF16, name=f"pt_{tb}_{g}_{k}", tag="pt")
                        nc.tensor.transpose(
                            pt, x0[:, g, k * 128 : (k + 1) * 128], ident
                        )
                        nc.vector.tensor_copy(
                            out=xT[:, k, g * 128 : (g + 1) * 128], in_=pt
                        )
                return {"xT": xT, "r_hi": r_hi, "r_lo": r_lo}

            def emit_gemm1(tb, tiles):
                hT = blk1.tile([128, MI, TB], BF16, name=f"hT_{tb}", tag="hT")
                for m in range(MI):
                    p1 = ps1.tile([128, TB], FP32, name=f"p1_{tb}_{m}", tag="p1")
                    for k in range(KH):
                        nc.tensor.matmul(
                            p1,
                            lhsT=w1_sb[:, k, m * 128 : (m + 1) * 128],
                            rhs=tiles["xT"][:, k, :],
                            start=(k == 0),
                            stop=(k == KH - 1),
                        )
                    nc.scalar.activation(
                        out=hT[:, m, :],
                        in_=p1,
                        func=AFT.Gelu_apprx_tanh,
                        bias=b1_sb[:, m : m + 1],
                        scale=1.0,
                    )
                tiles["hT"] = hT

            KS = 4

            def emit_g2n(tb, n, tiles):
                hT = tiles["hT"]
                p2s = [
                    ps2.tile([128, 512], FP32, name=f"p2_{tb}_{n}_{g}", tag="p2")
                    for g in range(G)
                ]
                for ks in range(MI // KS):
                    w2s = w2p.tile(
                        [128, KS, 512], BF16, name=f"w2s_{tb}_{n}_{ks}", tag="w2s"
                    )
                    src_ap = w2[
                        ks * KS * 128 : (ks + 1) * KS * 128,
                        n * 512 : (n + 1) * 512,
                    ].rearrange("(j p) c -> p j c", p=128)
                    nc.sync.dma_start(out=w2s, in_=src_ap)
                    for j in range(KS):
                        k = ks * KS + j
                        for g in range(G):
                            nc.tensor.matmul(
                                p2s[g],
                                lhsT=hT[:, k, g * 128 : (g + 1) * 128],
                                rhs=w2s[:, j, :],
                                start=(k == 0),
                                stop=(k == MI - 1),
                            )
                return p2s

            def emit_evict(tb, n, p2s, tiles):
                t0 = tb * TB
                for g in range(G):
                    o = outp.tile([128, 512], FP32, name=f"o_{tb}_{n}_{g}", tag="o")
                    nc.vector.tensor_add(
                        out=o,
                        in0=p2s[g],
                        in1=tiles["r_hi"][:, g, n * 512 : (n + 1) * 512],
                    )
                    nc.vector.tensor_add(
                        out=o,
                        in0=o,
                        in1=tiles["r_lo"][:, g, n * 512 : (n + 1) * 512],
                    )
                    nc.gpsimd.dma_start(
                        out=out[t0 + g * 128 : t0 + (g + 1) * 128, n * 512 : (n + 1) * 512],
                        in_=o,
                    )

            # Software-pipelined emission: block tb+1's LN/transposes are
            # emitted (and scheduled on DVE/PE) ahead of block tb's PSUM
            # eviction, so the PE never waits on the DVE catching up at a
            # block boundary.
            w1_sb = const.tile([128, KH, I], BF16)
            tiles = emit_ln(0)
            for k in range(KH):
                nc.sync.dma_start(out=w1_sb[:, k, :], in_=w1[k * 128 : (k + 1) * 128, :])
            for tb in range(n_blocks):
                emit_gemm1(tb, tiles)
                p2s0 = emit_g2n(tb, 0, tiles)
                next_tiles = emit_ln(tb + 1) if tb + 1 < n_blocks else None
                emit_evict(tb, 0, p2s0, tiles)
                p2s1 = emit_g2n(tb, 1, tiles)
                emit_evict(tb, 1, p2s1, tiles)
                tiles = next_tiles

    return nc


def _prep_inputs(input, residual, bias, attn_nw, attn_nb, inter_w, inter_b, output_w, output_b):
    """Host-side preprocessing: fold LN affine into W1/b1, cast weights to bf16,
    shard tokens across cores."""
    bf = ml_dtypes.bfloat16
    x2 = np.ascontiguousarray(np.asarray(input, np.float32).reshape(NTOK, H))
    r2 = np.ascontiguousarray(np.asarray(residual, np.float32).reshape(NTOK, H))
    gamma = np.asarray(attn_nw, np.float64)
    beta = np.asarray(attn_nb, np.float64)
    w1f = np.asarray(inter_w, np.float64)
    w1b = np.ascontiguousarray((gamma[:, None] * w1f).astype(np.float32).astype(bf))
    b1p = (np.asarray(inter_b, np.float64) + beta @ w1f).astype(np.float32)
    b1c = np.ascontiguousarray(b1p.reshape(MI, 128).T)
    w2b = np.ascontiguousarray(np.asarray(output_w, np.float32).astype(bf))
    biasf = np.asarray(bias, np.float32)
    b2f = np.asarray(output_b, np.float32)

    in_maps = []
    for c in range(N_CORES):
        sl = slice(c * T, (c + 1) * T)
        in_maps.append(
            {
                "xin": x2[sl],
                "res": r2[sl],
                "w1": w1b,
                "w2": w2b,
                "biasv": biasf,
                "b1c": b1c,
                "b2v": b2f,
            }
        )
    return in_maps


def _run(inputs, trace=False, **kwargs):
    in_maps = _prep_inputs(
        inputs["input"],
        inputs["residual"],
        inputs["bias"],
        inputs["attn_nw"],
        inputs["attn_nb"],
        inputs["inter_w"],
        inputs["inter_b"],
        inputs["output_w"],
        inputs["output_b"],
    )
    nc = _build()
    _split_multiwait_instructions(nc)
    r = run_bass_kernel_spmd(nc, in_maps, list(range(N_CORES)), trace=trace, **kwargs)
    outs = [r.results[c]["out"] for c in range(N_CORES)]
    full = np.concatenate(outs, axis=0).reshape(B, S, H).astype(np.float32)
    return full, r


def kernel(**inputs):
    out, _ = _run(inputs, trace=False)
    return out


if __name__ == "__main__":
    nc = _build(1)
    print("built 1-block variant ok:", len(nc.m.functions[0].blocks))



# revision 3
# speedup vs baseline: 1.2826x; 1.2826x over previous
"""DeepSpeed-style MLP block (pre-LN residual add + LN + GEMM+GELU + GEMM +
residual) for Trainium2, data-parallel over tokens across 8 NeuronCores.

fp8 (e4m3) DoubleRow variant: both GEMMs run with perf_mode=DoubleRow (2 fp8
weights per PE cell, K=256 per matmul) at ~1.7x the bf16 matmul rate. To keep
the fp8 quantization error well inside the 2e-2 gate, the GELU is split into a
linear part and a small nonlinear residual:

    h@W2 = g*@W2 + x@(W1'W2)/2,   g* = gelu(z) - (z - b1)/2

The fp8 stream carries only g* (~2.3x smaller than h, so ~2.3x less
quantization noise) while the linear half rides a bf16 GEMM against the
host-precomputed W12 = W1'@W2 (K=1024, 1/4 the FLOPs of GEMM2). The
W1-quantization noise similarly only enters through (gelu' - 1/2), not gelu'.
Host-side sim: rel err 1.05e-2 (vs 1.79e-2 for plain fp8, 1.1e-3 for bf16).

Per-core pipeline (tokens sharded 8 x 4096, processed in 512-token blocks):
  r   = input + bias + residual                      (fp32, DVE)
  x0  = 16 * (r - mean) * rsqrt(var + eps)           (bf16; LN affine folded
                                                      into W1/b1 on host)
  xT  = PE-transpose(x0)   -> bf16 copy (for W12 GEMM) + fp8 copy (for GEMM1)
  p1  = W1q-pairs.T @ xT_fp8     (DoubleRow, scale 16*1024)
  h   = gelu_tanh(p1/16384 + b1)                     (ACT, bf16)
  g*  = h - p1/32768                                 (DVE scalar_tensor_tensor,
                                                      written as fp8)
  p2  = sum_k g*-pairs.T @ W2q + sum_k xT_bf.T @ W12q   (DR fp8 + bf16, PSUM)
  out = p2/1024 + (r + output_b)                     (DVE stt, fp32)

W1/W2 (fp8, x1024) and W12 (bf16) are SBUF-resident; total DMA ~61 MB/core.
"""

import sys

sys.path.insert(0, "/opt/trn_rl_repo")

import numpy as np
import ml_dtypes

import concourse.bass as bass
import concourse.mybir as mybir
import concourse.tile as tile
from concourse.masks import make_identity
from concourse.bass_utils import run_bass_kernel_spmd

AFT = mybir.ActivationFunctionType
ALU = mybir.AluOpType
DR = mybir.MatmulPerfMode.DoubleRow
FP32 = mybir.dt.float32
BF16 = mybir.dt.bfloat16
FP8 = mybir.dt.float8e4

N_CORES = 8
B, S, H, I = 4, 8192, 1024, 4096
NTOK = B * S              # 32768 tokens total
T = NTOK // N_CORES       # 4096 tokens per core
TB = 512                  # tokens per block (moving free dim)
G = TB // 128             # 4 token sub-tiles per block
KH = H // 128             # 8 contraction chunks for GEMM1 / W12 GEMM
MI = I // 128             # 32 I-chunks (GEMM1 out / GEMM2 contraction)
NH = H // 512             # 2 H output slices for GEMM2
EPS = 1e-5

S_X = 16.0                # fp8 scale on the LN output x
S_W = 1024.0              # fp8 scale on W1 and W2
C1 = 1.0 / (S_X * S_W)    # GEMM1 psum -> z
USE_W12 = True            # scheme B (gelu split + bf16 W12 term)


def _split_multiwait_instructions(nc):
    """This walrus build accepts only ONE sync-wait command per instruction.
    Move extra waits onto fresh same-engine NOPs placed just before the
    offending instruction."""
    n_split = 0
    for f in nc.m.functions:
        for bb in f.blocks:
            insts = list(bb.instructions)
            new = []
            changed = False
            for inst in insts:
                si = inst.sync_info
                if si is not None and si.on_wait and len(si.on_wait) > 1:
                    waits = list(si.on_wait)
                    for w in waits[:-1]:
                        nop = mybir.InstNoOp(name=nc.get_next_instruction_name())
                        nop.engine = inst.engine
                        nop.sync_info = mybir.SyncInfo(on_wait=[w], on_update=[])
                        new.append(nop)
                        n_split += 1
                    si.on_wait = waits[-1:]
                    changed = True
                new.append(inst)
            if changed:
                bb.instructions = new
    return n_split


def _bcast_ap(ap, p=128):
    """AP view of a DRAM vector broadcast across p partitions."""
    return bass.AP(tensor=ap.tensor, offset=ap.offset, ap=[[0, p]] + list(ap.ap))


def _build(n_blocks=T // TB, use_w12=USE_W12):
    nc = bass.Bass("TRN2")
    t_rows = n_blocks * TB
    xin = nc.declare_dram_parameter("xin", [t_rows, H], FP32, isOutput=False)
    res = nc.declare_dram_parameter("res", [t_rows, H], FP32, isOutput=False)
    w1 = nc.declare_dram_parameter("w1", [H, I], FP8, isOutput=False)
    w2 = nc.declare_dram_parameter("w2", [I, H], FP8, isOutput=False)
    if use_w12:
        w12 = nc.declare_dram_parameter("w12", [H, H], BF16, isOutput=False)
    biasv = nc.declare_dram_parameter("biasv", [H], FP32, isOutput=False)
    b1c = nc.declare_dram_parameter("b1c", [128, MI], FP32, isOutput=False)
    b2v = nc.declare_dram_parameter("b2v", [H], FP32, isOutput=False)
    out = nc.declare_dram_parameter("out", [t_rows, H], FP32, isOutput=True)

    with tile.TileContext(nc) as tc:
        with (
            tc.tile_pool(name="const", bufs=1) as const,
            tc.tile_pool(name="ing", bufs=2) as ing,
            tc.tile_pool(name="tmpg", bufs=2) as tmpg,
            tc.tile_pool(name="blk1", bufs=1) as blk1,
            tc.tile_pool(name="blk2", bufs=2) as blk2,
            tc.tile_pool(name="htmp", bufs=3) as htmp,
            tc.tile_pool(name="outp", bufs=4) as outp,
            tc.tile_pool(name="statp", bufs=4) as statp,
            tc.tile_pool(name="ps1", bufs=2, space="PSUM") as ps1,
            tc.tile_pool(name="ps2", bufs=4, space="PSUM") as ps2,
            tc.tile_pool(name="pst", bufs=2, space="PSUM") as pst,
        ):
            # ---- preload constants (small consts first: the LN chain needs
            # bias_bc immediately; the weights are needed once GEMM1/GEMM2 of
            # block 0 start and are DMA'd after block 0's input loads) ----
            bias_bc = const.tile([128, H], FP32)
            nc.gpsimd.dma_start(out=bias_bc, in_=_bcast_ap(biasv[:]))
            b2_bc = const.tile([128, H], FP32)
            nc.gpsimd.dma_start(out=b2_bc, in_=_bcast_ap(b2v[:]))
            b1_sb = const.tile([128, MI], FP32)
            nc.gpsimd.dma_start(out=b1_sb, in_=b1c[:, :])
            eps_t = const.tile([128, 1], FP32)
            nc.vector.memset(eps_t, EPS / (S_X * S_X))
            ident = const.tile([128, 128], BF16)
            make_identity(nc, ident)

            def emit_ln(tb):
                """Load + pre-LN + LN + PE-transpose for block tb.
                Returns the block tiles used by the GEMM/evict stages."""
                t0 = tb * TB
                x0 = blk1.tile([128, G, H], BF16, name=f"x0_{tb}", tag="x0")
                xT8 = blk1.tile([128, KH, TB], FP8, name=f"xT8_{tb}", tag="xT8")
                xTb = (
                    blk1.tile([128, KH, TB], BF16, name=f"xTb_{tb}", tag="xTb")
                    if use_w12
                    else None
                )
                r32 = blk2.tile([128, G, H], FP32, name=f"r32_{tb}", tag="r32")
                for g in range(G):
                    ra, rb = t0 + g * 128, t0 + (g + 1) * 128
                    xin_g = ing.tile([128, H], FP32, name=f"xin_{tb}_{g}", tag="xin")
                    res_g = ing.tile([128, H], FP32, name=f"res_{tb}_{g}", tag="res")
                    nc.sync.dma_start(out=xin_g, in_=xin[ra:rb, :])
                    nc.sync.dma_start(out=res_g, in_=res[ra:rb, :])
                    tmp = tmpg.tile([128, H], FP32, name=f"tmp_{tb}_{g}", tag="tmp")
                    nc.vector.tensor_add(out=tmp, in0=xin_g, in1=res_g)
                    nc.vector.tensor_add(out=tmp, in0=tmp, in1=bias_bc)
                    stats = statp.tile([128, 2, 6], FP32, name=f"st_{tb}_{g}", tag="stats")
                    tmp_r = tmp.rearrange("p (s d) -> p s d", s=2)
                    for s_ in range(2):
                        nc.vector.bn_stats(out=stats[:, s_, :], in_=tmp_r[:, s_, :])
                    mv = statp.tile([128, 2], FP32, name=f"mv_{tb}_{g}", tag="mv")
                    nc.vector.bn_aggr(out=mv, in_=stats)
                    rstd = statp.tile([128, 1], FP32, name=f"rs_{tb}_{g}", tag="rstd")
                    # sqrt((var+eps)/S_X^2) then reciprocal -> S_X * rsqrt(var+eps)
                    nc.scalar.activation(
                        out=rstd, in_=mv[:, 1:2], func=AFT.Sqrt, bias=eps_t,
                        scale=1.0 / (S_X * S_X),
                    )
                    nc.vector.reciprocal(out=rstd, in_=rstd)
                    nc.vector.tensor_scalar(
                        out=x0[:, g, :],
                        in0=tmp,
                        scalar1=mv[:, 0:1],
                        scalar2=rstd,
                        op0=ALU.subtract,
                        op1=ALU.mult,
                    )
                    nc.vector.tensor_add(out=r32[:, g, :], in0=tmp, in1=b2_bc)
                    for k in range(KH):
                        pt = pst.tile([128, 128], BF16, name=f"pt_{tb}_{g}_{k}", tag="pt")
                        nc.tensor.transpose(
                            pt, x0[:, g, k * 128 : (k + 1) * 128], ident
                        )
                        if use_w12:
                            nc.vector.tensor_copy(
                                out=xTb[:, k, g * 128 : (g + 1) * 128], in_=pt
                            )
                        nc.vector.tensor_copy(
                            out=xT8[:, k, g * 128 : (g + 1) * 128], in_=pt
                        )
                return {"xT8": xT8, "xTb": xTb, "r32": r32}

            def emit_gemm1(tb, tiles):
                hT = blk1.tile([128, MI, TB], FP8, name=f"hT_{tb}", tag="hT")
                for m in range(MI):
                    p1 = ps1.tile([128, TB], FP32, name=f"p1_{tb}_{m}", tag="p1")
                    for k in range(KH // 2):
                        nc.tensor.matmul(
                            p1,
                            lhsT=w1_sb[:, 2 * k : 2 * k + 2, m * 128 : (m + 1) * 128],
                            rhs=tiles["xT8"][:, 2 * k : 2 * k + 2, :],
                            start=(k == 0),
                            stop=(k == KH // 2 - 1),
                            perf_mode=DR,
                        )
                    if use_w12:
                        # h = gelu(p1*C1 + b1); g* = h - (C1/2)*p1, stored fp8
                        h_t = htmp.tile([128, TB], BF16, name=f"ht_{tb}_{m}", tag="ht")
                        nc.scalar.activation(
                            out=h_t,
                            in_=p1,
                            func=AFT.Gelu_apprx_tanh,
                            bias=b1_sb[:, m : m + 1],
                            scale=C1,
                        )
                        nc.vector.scalar_tensor_tensor(
                            out=hT[:, m, :],
                            in0=p1,
                            scalar=-C1 / 2,
                            in1=h_t,
                            op0=ALU.mult,
                            op1=ALU.add,
                        )
                    else:
                        nc.scalar.activation(
                            out=hT[:, m, :],
                            in_=p1,
                            func=AFT.Gelu_apprx_tanh,
                            bias=b1_sb[:, m : m + 1],
                            scale=C1,
                        )
                tiles["hT"] = hT

            def emit_g2n(tb, n, tiles):
                hT = tiles["hT"]
                p2s = [
                    ps2.tile([128, 512], FP32, name=f"p2_{tb}_{n}_{g}", tag="p2")
                    for g in range(G)
                ]
                if use_w12:
                    xTb = tiles["xTb"]
                    for k in range(KH):
                        for g in range(G):
                            nc.tensor.matmul(
                                p2s[g],
                                lhsT=xTb[:, k, g * 128 : (g + 1) * 128],
                                rhs=w12_sb[:, k, n * 512 : (n + 1) * 512],
                                start=(k == 0),
                                stop=False,
                            )
                for k in range(MI // 2):
                    for g in range(G):
                        nc.tensor.matmul(
                            p2s[g],
                            lhsT=hT[:, 2 * k : 2 * k + 2, g * 128 : (g + 1) * 128],
                            rhs=w2_sb[:, 2 * k : 2 * k + 2, n * 512 : (n + 1) * 512],
                            start=(not use_w12) and (k == 0),
                            stop=(k == MI // 2 - 1),
                            perf_mode=DR,
                        )
                return p2s

            def emit_evict(tb, n, p2s, tiles):
                t0 = tb * TB
                for g in range(G):
                    o = outp.tile([128, 512], FP32, name=f"o_{tb}_{n}_{g}", tag="o")
                    nc.vector.scalar_tensor_tensor(
                        out=o,
                        in0=p2s[g],
                        scalar=1.0 / S_W,
                        in1=tiles["r32"][:, g, n * 512 : (n + 1) * 512],
                        op0=ALU.mult,
                        op1=ALU.add,
                    )
                    nc.gpsimd.dma_start(
                        out=out[t0 + g * 128 : t0 + (g + 1) * 128, n * 512 : (n + 1) * 512],
                        in_=o,
                    )

            # Software-pipelined emission: block tb+1's LN/transposes are
            # emitted (and scheduled on DVE/PE) ahead of block tb's PSUM
            # eviction, so the PE never waits on the DVE catching up at a
            # block boundary.
            w1_sb = const.tile([128, KH, I], FP8, name="w1_sb")
            w2_sb = const.tile([128, MI, H], FP8, name="w2_sb")
            w12_sb = (
                const.tile([128, KH, H], BF16, name="w12_sb") if use_w12 else None
            )
            tiles = emit_ln(0)
            for k in range(KH):
                nc.sync.dma_start(out=w1_sb[:, k, :], in_=w1[k * 128 : (k + 1) * 128, :])
            for ks in range(4):
                nc.sync.dma_start(
                    out=w2_sb[:, ks * 8 : (ks + 1) * 8, :],
                    in_=w2[ks * 8 * 128 : (ks + 1) * 8 * 128, :].rearrange(
                        "(k p) h -> p k h", p=128
                    ),
                )
            if use_w12:
                nc.sync.dma_start(
                    out=w12_sb,
                    in_=w12[:, :].rearrange("(k p) h -> p k h", p=128),
                )
            for tb in range(n_blocks):
                emit_gemm1(tb, tiles)
                p2s0 = emit_g2n(tb, 0, tiles)
                next_tiles = emit_ln(tb + 1) if tb + 1 < n_blocks else None
                emit_evict(tb, 0, p2s0, tiles)
                p2s1 = emit_g2n(tb, 1, tiles)
                emit_evict(tb, 1, p2s1, tiles)
                tiles = next_tiles

    return nc


def _prep_inputs(input, residual, bias, attn_nw, attn_nb, inter_w, inter_b, output_w, output_b, use_w12=USE_W12):
    """Host-side preprocessing: fold LN affine into W1/b1, scale + cast weights
    to fp8 e4m3 (clip to +-240: TRN e4m3 overflows to inf), precompute
    W12 = W1'@W2 in bf16, shard tokens across cores."""
    f8 = ml_dtypes.float8_e4m3
    bf = ml_dtypes.bfloat16
    x2 = np.ascontiguousarray(np.asarray(input, np.float32).reshape(NTOK, H))
    r2 = np.ascontiguousarray(np.asarray(residual, np.float32).reshape(NTOK, H))
    gamma = np.asarray(attn_nw, np.float64)
    beta = np.asarray(attn_nb, np.float64)
    w1f = np.asarray(inter_w, np.float64)
    w2f = np.asarray(output_w, np.float64)
    w1p = gamma[:, None] * w1f
    w1b = np.ascontiguousarray(
        np.clip(w1p * S_W, -240, 240).astype(np.float32).astype(f8)
    )
    b1p = (np.asarray(inter_b, np.float64) + beta @ w1f).astype(np.float32)
    b1c = np.ascontiguousarray(b1p.reshape(MI, 128).T)
    w2b = np.ascontiguousarray(
        np.clip(w2f * S_W, -240, 240).astype(np.float32).astype(f8)
    )
    biasf = np.asarray(bias, np.float32)
    b2f = np.asarray(output_b, np.float32)

    in_maps = []
    for c in range(N_CORES):
        sl = slice(c * T, (c + 1) * T)
        im = {
            "xin": x2[sl],
            "res": r2[sl],
            "w1": w1b,
            "w2": w2b,
            "biasv": biasf,
            "b1c": b1c,
            "b2v": b2f,
        }
        in_maps.append(im)
    if use_w12:
        w12b = np.ascontiguousarray(
            ((w1p @ w2f) * (S_W / (2 * S_X))).astype(np.float32).astype(bf)
        )
        for im in in_maps:
            im["w12"] = w12b
    return in_maps


def _run(inputs, trace=False, **kwargs):
    in_maps = _prep_inputs(
        inputs["input"],
        inputs["residual"],
        inputs["bias"],
        inputs["attn_nw"],
        inputs["attn_nb"],
        inputs["inter_w"],
        inputs["inter_b"],
        inputs["output_w"],
        inputs["output_b"],
    )
    nc = _build()
    _split_multiwait_instructions(nc)
    r = run_bass_kernel_spmd(nc, in_maps, list(range(N_CORES)), trace=trace, **kwargs)
    outs = [r.results[c]["out"] for c in range(N_CORES)]
    full = np.concatenate(outs, axis=0).reshape(B, S, H).astype(np.float32)
    return full, r


def kernel(**inputs):
    out, _ = _run(inputs, trace=False)
    return out


if __name__ == "__main__":
    nc = _build(1)
    print("built 1-block variant ok:", len(nc.m.functions[0].blocks))
